# revision 1
# baseline (speedup 1.0000x reference)
"""CrossMHA Trainium2 kernel (8 NeuronCores, data-parallel batch x q-half).

Reference computation (b=4, ql=kl=1024, DIM=1024, H=16, dk=64):
    qs  = decoder @ Wq.T                     [b, q, 1024]
    kv  = encoder @ Wkv.T ; ks, vs = split   [b, k, 1024] each
    head-LAST reshape: channel c = d*16 + h  (d in 0..63, h in 0..15)
    w   = softmax((qs . ks)/8 over k)        [b, q, k, h]   (mask is all-ones)
    vals = (w . vs)  -> flatten -> @ Wout.T @ Wout.T

Sharding: 8 cores = 4 batches x 2 q-halves of 512. Each core computes the
full K/V projection for its batch (duplicated across the q-pair) and its
own q-slice of everything else. No collectives.

Device layout: all activations are feature-major ("transposed", channels on
partitions), so attention needs no on-device transposes:
    xT = decoder[bi].T[:, qslice]   [1024, 512]
    eT = encoder[bi].T              [1024, 1024]
Weights are pre-permuted on the host so each head's 64 channels are
contiguous (perm[h*64+d] = d*16+h), and pre-transposed to [in, out] so they
are direct matmul lhsT slices.

Projections and out-projections run in float32r (4x fp32 PE speed, ~1.5e-4
rel err). Attention probabilities and V run in bf16 (same PE speed, halves
SBUF so exp can double-buffer across heads). Softmax sums come free from a
ones-column appended to V (AV output row 64); normalization broadcasts 1/s
across partitions via a K=1 matmul.

Phase order pipelines ACT exp under PE projection work:
    q-proj, v-proj, then per head-pair ct: k-proj[ct] -> scores -> exp ->
    AV -> normalize, then out1, out2.
"""
import sys

sys.path.insert(0, "/opt/trn_rl_repo")

import numpy as np

import concourse.bacc as bacc
import concourse.tile as tile
from concourse import mybir
from concourse.bass_utils import run_bass_kernel_spmd

F32 = mybir.dt.float32
F32R = mybir.dt.float32r
BF16 = mybir.dt.bfloat16
EXP = mybir.ActivationFunctionType.Exp

DIM = 1024
H = 16
DK = 64
QT = 512          # q rows per core
IT = DIM // 128   # 8 tiles of 128 along any 1024 dim

import os as _os
BF16_PROJ = _os.environ.get("KERNEL_BF16_PROJ", "0") == "1"

_CACHE = {}


def build_nc():
    nc = bacc.Bacc("TRN2", target_bir_lowering=False, debug=False, num_devices=8)
    PDT = BF16 if BF16_PROJ else F32
    xT = nc.dram_tensor("xT", [DIM, QT], PDT, kind="ExternalInput").ap()
    eT = nc.dram_tensor("eT", [DIM, DIM], PDT, kind="ExternalInput").ap()
    wqT = nc.dram_tensor("wqT", [DIM, DIM], PDT, kind="ExternalInput").ap()
    wkT = nc.dram_tensor("wkT", [DIM, DIM], PDT, kind="ExternalInput").ap()
    wvT = nc.dram_tensor("wvT", [DIM, DIM], PDT, kind="ExternalInput").ap()
    wo1T = nc.dram_tensor("wo1T", [DIM, DIM], F32, kind="ExternalInput").ap()
    wo2T = nc.dram_tensor("wo2T", [DIM, DIM], F32, kind="ExternalInput").ap()
    onesA = nc.dram_tensor("onesA", [128, H], BF16, kind="ExternalInput").ap()
    onesB = nc.dram_tensor("onesB", [1, 64], F32, kind="ExternalInput").ap()
    outT = nc.dram_tensor("outT", [DIM, QT], F32, kind="ExternalOutput").ap()

    from contextlib import ExitStack
    with tile.TileContext(nc) as tc, ExitStack() as ctx:
        build_tile(ctx, tc, nc, xT, eT, wqT, wkT, wvT, wo1T, wo2T, onesA, onesB, outT)
    nc.compile()
    return nc


def build_tile(ctx, tc, nc, xT, eT, wqT, wkT, wvT, wo1T, wo2T, onesA, onesB, outT):
    p_t2k = ctx.enter_context(tc.tile_pool(name="t2k", bufs=8))   # xT then valsT
    p_e = ctx.enter_context(tc.tile_pool(name="e", bufs=8))
    p_w = ctx.enter_context(tc.tile_pool(name="w", bufs=14))
    p_qs = ctx.enter_context(tc.tile_pool(name="qs", bufs=8))
    p_ks = ctx.enter_context(tc.tile_pool(name="ks", bufs=8))    # ksT then out1T/outT
    p_vs = ctx.enter_context(tc.tile_pool(name="vs", bufs=8))
    p_exp = ctx.enter_context(tc.tile_pool(name="exp", bufs=16))
    p_sm = ctx.enter_context(tc.tile_pool(name="sm", bufs=4))
    ps_a = ctx.enter_context(tc.tile_pool(name="psa", bufs=4, space="PSUM"))
    ps_v = ctx.enter_context(tc.tile_pool(name="psv", bufs=2, space="PSUM"))
    ps_r = ctx.enter_context(tc.tile_pool(name="psr", bufs=2, space="PSUM"))

    # ---- ones tiles (DMA'd from host: memset cannot produce f32r/rounded) ----
    onesT = p_sm.tile([128, H], BF16, tag="onesT", bufs=1)
    nc.sync.dma_start(out=onesT[:], in_=onesA)
    ones64 = p_sm.tile([1, 64], F32R, tag="ones64", bufs=1)
    nc.sync.dma_start(out=ones64[:], in_=onesB.bitcast(F32R))

    # ---- loads ----
    # n_split > 1 issues column-chunk DMAs in chunk-major order so consumers
    # that read column slices (every projection's lhsT) can start as soon as
    # their columns land (Tile tracks subtile deps).
    def load(pool, src, cols, tag, n_split=1, dt=F32R):
        ts = [pool.tile([128, cols], dt, tag=tag, name=f"{tag}{ic}")
              for ic in range(IT)]
        w = cols // n_split
        for sp in range(n_split):
            for ic in range(IT):
                nc.sync.dma_start(
                    out=ts[ic][:, sp * w:(sp + 1) * w],
                    in_=src[ic * 128:(ic + 1) * 128, sp * w:(sp + 1) * w].bitcast(dt))
        return ts

    PDTR = BF16 if BF16_PROJ else F32R
    x_t = load(p_t2k, xT, QT, "t2k", dt=PDTR)
    wq_t = load(p_w, wqT, DIM, "w", n_split=2, dt=PDTR)
    e_t = load(p_e, eT, DIM, "e", dt=PDTR)
    wv_t = load(p_w, wvT, DIM, "w", dt=PDTR)
    wk_t = load(p_w, wkT, DIM, "w", dt=PDTR)

    # ---- Q projection: qsT[c, q] ----
    qs_t = []
    for ct in range(IT):
        ps = ps_a.tile([128, QT], F32, tag="psa", name=f"psq{ct}")
        for ic in range(IT):
            nc.tensor.matmul(ps[:], wq_t[ic][:, ct * 128:(ct + 1) * 128], x_t[ic][:],
                             start=(ic == 0), stop=(ic == IT - 1))
        t = p_qs.tile([128, QT], F32R, tag="qs", name=f"qs{ct}")
        nc.vector.tensor_copy(t[:], ps[:])
        qs_t.append(t)

    # ---- V projection: vs[k, c] in bf16, 65 cols/head (col 64 = ones) ----
    vs_t = []
    for kt in range(IT):
        t = p_vs.tile([128, H * 65], BF16, tag="vs", name=f"vs{kt}")
        for nt in range(2):
            ps = ps_a.tile([128, QT], F32, tag="psa", name=f"psvp{kt}_{nt}")
            for ic in range(IT):
                nc.tensor.matmul(ps[:], e_t[ic][:, kt * 128:(kt + 1) * 128],
                                 wv_t[ic][:, nt * 512:(nt + 1) * 512],
                                 start=(ic == 0), stop=(ic == IT - 1))
            src = ps[:].rearrange("p (h d) -> p h d", d=64)
            dst = t[:, nt * 520:(nt + 1) * 520].rearrange("p (h e) -> p h e", e=65)
            nc.vector.tensor_copy(dst[:, :, 0:64], src)
        ocol = t[:].rearrange("p (h e) -> p h e", e=65)
        nc.vector.tensor_copy(ocol[:, :, 64:65],
                              onesT[:].rearrange("p (h o) -> p h o", o=1))
        vs_t.append(t)

    # out-projection weights stream in as slots free up
    wo1_t = load(p_w, wo1T, DIM, "w")
    wo2_t = load(p_w, wo2T, DIM, "w")

    # ---- K projection + attention, pipelined per head-pair ct ----
    val_t = []
    pending = []  # deferred normalize: (vt, po, ps_av, r)

    def finalize(p):
        vt, po, ps_av, r = p
        ps_b = ps_r.tile([64, QT], F32, tag="psr", name="psb")
        nc.tensor.matmul(ps_b[:], ones64[:], r[:], start=True, stop=True)
        nc.vector.tensor_copy(vt[po:po + 64, :], ps_av[0:64, :])
        nc.vector.tensor_mul(vt[po:po + 64, :], vt[po:po + 64, :], ps_b[:])

    for ct in range(IT):
        # ksT[c, k] for this head pair
        kst = p_ks.tile([128, DIM], F32R, tag="ks", name=f"ks{ct}")
        for nt in range(2):
            ps = ps_a.tile([128, QT], F32, tag="psa", name=f"pskp{ct}_{nt}")
            for ic in range(IT):
                nc.tensor.matmul(ps[:], wk_t[ic][:, ct * 128:(ct + 1) * 128],
                                 e_t[ic][:, nt * 512:(nt + 1) * 512],
                                 start=(ic == 0), stop=(ic == IT - 1))
            nc.vector.tensor_copy(kst[:, nt * 512:(nt + 1) * 512], ps[:])

        vt = p_t2k.tile([128, QT], F32R, tag="t2k", name=f"val{ct}")
        # scores + exp for both heads, kt-major: the sub=0 (rows 0:64) and
        # sub=1 (rows 64:128) matmuls sit on disjoint PE row-groups and
        # different PSUM banks, so adjacent pairs execute concurrently.
        exps = {0: [], 1: []}
        import os
        if os.environ.get("KERNEL_SC_INTERLEAVE", "1") == "1":
            order = [(kt, sub) for kt in range(IT) for sub in range(2)]
        else:
            order = [(kt, sub) for sub in range(2) for kt in range(IT)]
        for kt, sub in order:
            h = ct * 2 + sub
            po = sub * 64
            ps_s = ps_a.tile([128, QT], F32, tag="psa", name=f"pss{h}_{kt}")
            nc.tensor.matmul(ps_s[:], kst[po:po + 64, kt * 128:(kt + 1) * 128],
                             qs_t[ct][po:po + 64, :], start=True, stop=True)
            et = p_exp.tile([128, QT], BF16, tag="exp", name=f"ex{h}_{kt}")
            nc.scalar.activation(et[:], ps_s[:], EXP, scale=0.125)
            exps[sub].append(et)
        for sub in range(2):
            h = ct * 2 + sub
            po = sub * 64
            ps_av = ps_v.tile([128, QT], F32, tag="psv", name=f"psav{h}")
            for kt in range(IT):
                nc.tensor.matmul(ps_av[0:65, :], vs_t[kt][:, h * 65:(h + 1) * 65],
                                 exps[sub][kt][:], start=(kt == 0), stop=(kt == IT - 1))
            r = p_sm.tile([1, QT], F32R, tag="r", name=f"r{h}", bufs=2)
            with nc.allow_low_precision(reason="1/s rounded to f32r for bcast matmul"):
                nc.vector.reciprocal(r[:], ps_av[64:65, :])
            if pending:
                finalize(pending.pop(0))
            pending.append((vt, po, ps_av, r))
        val_t.append(vt)
    while pending:
        finalize(pending.pop(0))

    # ---- out1 = Wout_p . valsT ; out2 = Wout . out1T ----
    o1_t = []
    for ot in range(IT):
        ps = ps_a.tile([128, QT], F32, tag="psa", name=f"pso1_{ot}")
        for ic in range(IT):
            nc.tensor.matmul(ps[:], wo1_t[ic][:, ot * 128:(ot + 1) * 128], val_t[ic][:],
                             start=(ic == 0), stop=(ic == IT - 1))
        t = p_ks.tile([128, QT], F32R, tag="ks", name=f"o1_{ot}")
        nc.vector.tensor_copy(t[:], ps[:])
        o1_t.append(t)

    for ot in range(IT):
        ps = ps_a.tile([128, QT], F32, tag="psa", name=f"pso2_{ot}")
        for ic in range(IT):
            nc.tensor.matmul(ps[:], wo2_t[ic][:, ot * 128:(ot + 1) * 128], o1_t[ic][:],
                             start=(ic == 0), stop=(ic == IT - 1))
        t = p_qs.tile([128, QT], F32, tag="qs", name=f"ot{ot}")
        nc.vector.tensor_copy(t[:], ps[:])
        nc.sync.dma_start(out=outT[ot * 128:(ot + 1) * 128, :], in_=t[:])


def _prep(Wq, Wkv, Wout):
    """Host-side weight permutation/transposition (all fp32 numpy)."""
    d = np.arange(DK)
    h = np.arange(H)
    # perm[h*64+d] = d*16+h
    perm = (d[None, :] * H + h[:, None]).reshape(-1)
    Wk = Wkv[:DIM]
    Wv = Wkv[DIM:]
    wqT = np.ascontiguousarray(Wq[perm, :].T)
    wkT = np.ascontiguousarray(Wk[perm, :].T)
    wvT = np.ascontiguousarray(Wv[perm, :].T)
    wo1T = np.ascontiguousarray(Wout[:, perm].T)
    wo2T = np.ascontiguousarray(Wout.T)
    return wqT, wkT, wvT, wo1T, wo2T


def kernel(decoder_input, encoder_input, cross_mask, Wq, Wkv, Wout, _trace=False):
    import ml_dtypes
    decoder_input = np.asarray(decoder_input, dtype=np.float32)
    encoder_input = np.asarray(encoder_input, dtype=np.float32)
    Wq = np.asarray(Wq, dtype=np.float32)
    Wkv = np.asarray(Wkv, dtype=np.float32)
    Wout = np.asarray(Wout, dtype=np.float32)
    b, ql, _ = decoder_input.shape

    if "nc" not in _CACHE:
        _CACHE["nc"] = build_nc()
    nc = _CACHE["nc"]

    wqT, wkT, wvT, wo1T, wo2T = _prep(Wq, Wkv, Wout)
    pdt = ml_dtypes.bfloat16 if BF16_PROJ else np.float32
    wqT, wkT, wvT = wqT.astype(pdt), wkT.astype(pdt), wvT.astype(pdt)
    in_maps = []
    for core in range(8):
        bi, qh = divmod(core, 2)
        xT = np.ascontiguousarray(decoder_input[bi].T[:, qh * QT:(qh + 1) * QT]).astype(pdt)
        eT = np.ascontiguousarray(encoder_input[bi].T).astype(pdt)
        in_maps.append({"xT": xT, "eT": eT, "wqT": wqT, "wkT": wkT, "wvT": wvT,
                        "wo1T": wo1T, "wo2T": wo2T,
                        "onesA": np.ones((128, H), ml_dtypes.bfloat16),
                        "onesB": np.ones((1, 64), np.float32)})

    _CACHE["in_maps"] = in_maps
    res = run_bass_kernel_spmd(nc, in_maps, list(range(8)), trace=_trace)
    out = np.empty((b, ql, DIM), dtype=np.float32)
    for core in range(8):
        bi, qh = divmod(core, 2)
        out[bi, qh * QT:(qh + 1) * QT, :] = res.results[core]["outT"].T
    if _trace:
        _CACHE["last_result"] = res
    return out



# revision 2
# speedup vs baseline: 10.2154x; 10.2154x over previous
"""CrossMHA Trainium2 kernel (8 NeuronCores, data-parallel batch x q-half).

Reference computation (b=4, ql=kl=1024, DIM=1024, H=16, dk=64):
    qs  = decoder @ Wq.T                     [b, q, 1024]
    kv  = encoder @ Wkv.T ; ks, vs = split   [b, k, 1024] each
    head-LAST reshape: channel c = d*16 + h  (d in 0..63, h in 0..15)
    w   = softmax((qs . ks)/8 over k)        [b, q, k, h]   (mask is all-ones)
    vals = (w . vs)  -> flatten -> @ Wout.T @ Wout.T

Sharding: 8 cores = 4 batches x 2 q-halves of 512. Each core computes the
full K/V projection for its batch (duplicated across the q-pair) and its
own q-slice of everything else. No collectives.

Device layout: all activations are feature-major ("transposed", channels on
partitions), so attention needs no on-device transposes:
    xT = decoder[bi].T[:, qslice]   [1024, 512]
    eT = encoder[bi].T              [1024, 1024]
Weights are pre-permuted on the host so each head's 64 channels are
contiguous (perm[h*64+d] = d*16+h), and pre-transposed to [in, out] so they
are direct matmul lhsT slices.

Projections and out-projections run in float32r (4x fp32 PE speed, ~1.5e-4
rel err). Attention probabilities and V run in bf16 (same PE speed, halves
SBUF so exp can double-buffer across heads). Softmax sums come free from a
ones-column appended to V (AV output row 64); normalization broadcasts 1/s
across partitions via a K=1 matmul.

Phase order pipelines ACT exp under PE projection work:
    q-proj, v-proj, then per head-pair ct: k-proj[ct] -> scores -> exp ->
    AV -> normalize, then out1, out2.
"""
import sys

sys.path.insert(0, "/opt/trn_rl_repo")

import numpy as np

import concourse.bacc as bacc
import concourse.tile as tile
from concourse import mybir
from concourse.bass_utils import run_bass_kernel_spmd

F32 = mybir.dt.float32
F32R = mybir.dt.float32r
BF16 = mybir.dt.bfloat16
EXP = mybir.ActivationFunctionType.Exp

DIM = 1024
H = 16
DK = 64
QT = 512          # q rows per core
IT = DIM // 128   # 8 tiles of 128 along any 1024 dim

import os as _os
BF16_PROJ = _os.environ.get("KERNEL_BF16_PROJ", "0") == "1"

_CACHE = {}


def build_nc(reps=1):
    """reps>1 repeats the whole kernel body inside one NEFF (used by test.py
    to measure per-execution HW time with dispatch overhead amortized)."""
    nc = bacc.Bacc("TRN2", target_bir_lowering=False, debug=False, num_devices=8)
    PDT = BF16 if BF16_PROJ else F32
    xT = nc.dram_tensor("xT", [DIM, QT], PDT, kind="ExternalInput").ap()
    eT = nc.dram_tensor("eT", [DIM, DIM], PDT, kind="ExternalInput").ap()
    wqT = nc.dram_tensor("wqT", [DIM, DIM], PDT, kind="ExternalInput").ap()
    wkT = nc.dram_tensor("wkT", [DIM, DIM], PDT, kind="ExternalInput").ap()
    wvT = nc.dram_tensor("wvT", [DIM, DIM], PDT, kind="ExternalInput").ap()
    wo1T = nc.dram_tensor("wo1T", [DIM, DIM], F32, kind="ExternalInput").ap()
    wo2T = nc.dram_tensor("wo2T", [DIM, DIM], F32, kind="ExternalInput").ap()
    onesA = nc.dram_tensor("onesA", [128, H], BF16, kind="ExternalInput").ap()
    onesB = nc.dram_tensor("onesB", [1, 64], F32, kind="ExternalInput").ap()
    outT = nc.dram_tensor("outT", [DIM, QT], F32, kind="ExternalOutput").ap()

    from contextlib import ExitStack
    with tile.TileContext(nc) as tc:
        for _ in range(reps):
            with ExitStack() as ctx:
                build_tile(ctx, tc, nc, xT, eT, wqT, wkT, wvT, wo1T, wo2T,
                           onesA, onesB, outT)
    nc.compile()
    return nc


def build_tile(ctx, tc, nc, xT, eT, wqT, wkT, wvT, wo1T, wo2T, onesA, onesB, outT):
    p_t2k = ctx.enter_context(tc.tile_pool(name="t2k", bufs=8))   # xT then valsT
    p_e = ctx.enter_context(tc.tile_pool(name="e", bufs=8))
    p_w = ctx.enter_context(tc.tile_pool(name="w", bufs=14))
    p_qs = ctx.enter_context(tc.tile_pool(name="qs", bufs=8))
    p_ks = ctx.enter_context(tc.tile_pool(name="ks", bufs=8))    # ksT then out1T/outT
    p_vs = ctx.enter_context(tc.tile_pool(name="vs", bufs=8))
    p_exp = ctx.enter_context(tc.tile_pool(name="exp", bufs=16))
    p_sm = ctx.enter_context(tc.tile_pool(name="sm", bufs=4))
    ps_a = ctx.enter_context(tc.tile_pool(name="psa", bufs=4, space="PSUM"))
    ps_v = ctx.enter_context(tc.tile_pool(name="psv", bufs=2, space="PSUM"))
    ps_r = ctx.enter_context(tc.tile_pool(name="psr", bufs=2, space="PSUM"))

    # ---- ones tiles (DMA'd from host: memset cannot produce f32r/rounded) ----
    onesT = p_sm.tile([128, H], BF16, tag="onesT", bufs=1)
    nc.sync.dma_start(out=onesT[:], in_=onesA)
    ones64 = p_sm.tile([1, 64], F32R, tag="ones64", bufs=1)
    nc.sync.dma_start(out=ones64[:], in_=onesB.bitcast(F32R))

    # ---- loads ----
    # n_split > 1 issues column-chunk DMAs in chunk-major order so consumers
    # that read column slices (every projection's lhsT) can start as soon as
    # their columns land (Tile tracks subtile deps).
    def load(pool, src, cols, tag, n_split=1, dt=F32R):
        ts = [pool.tile([128, cols], dt, tag=tag, name=f"{tag}{ic}")
              for ic in range(IT)]
        w = cols // n_split
        for sp in range(n_split):
            for ic in range(IT):
                nc.sync.dma_start(
                    out=ts[ic][:, sp * w:(sp + 1) * w],
                    in_=src[ic * 128:(ic + 1) * 128, sp * w:(sp + 1) * w].bitcast(dt))
        return ts

    PDTR = BF16 if BF16_PROJ else F32R
    x_t = load(p_t2k, xT, QT, "t2k", dt=PDTR)
    wq_t = load(p_w, wqT, DIM, "w", n_split=2, dt=PDTR)
    e_t = load(p_e, eT, DIM, "e", dt=PDTR)
    wv_t = load(p_w, wvT, DIM, "w", dt=PDTR)
    wk_t = load(p_w, wkT, DIM, "w", dt=PDTR)

    # ---- Q projection: qsT[c, q] ----
    qs_t = []
    for ct in range(IT):
        ps = ps_a.tile([128, QT], F32, tag="psa", name=f"psq{ct}")
        for ic in range(IT):
            nc.tensor.matmul(ps[:], wq_t[ic][:, ct * 128:(ct + 1) * 128], x_t[ic][:],
                             start=(ic == 0), stop=(ic == IT - 1))
        t = p_qs.tile([128, QT], F32R, tag="qs", name=f"qs{ct}")
        nc.vector.tensor_copy(t[:], ps[:])
        qs_t.append(t)

    # ---- V projection: vs[k, c] in bf16, 65 cols/head (col 64 = ones) ----
    vs_t = []
    for kt in range(IT):
        t = p_vs.tile([128, H * 65], BF16, tag="vs", name=f"vs{kt}")
        for nt in range(2):
            ps = ps_a.tile([128, QT], F32, tag="psa", name=f"psvp{kt}_{nt}")
            for ic in range(IT):
                nc.tensor.matmul(ps[:], e_t[ic][:, kt * 128:(kt + 1) * 128],
                                 wv_t[ic][:, nt * 512:(nt + 1) * 512],
                                 start=(ic == 0), stop=(ic == IT - 1))
            src = ps[:].rearrange("p (h d) -> p h d", d=64)
            dst = t[:, nt * 520:(nt + 1) * 520].rearrange("p (h e) -> p h e", e=65)
            nc.vector.tensor_copy(dst[:, :, 0:64], src)
        ocol = t[:].rearrange("p (h e) -> p h e", e=65)
        nc.vector.tensor_copy(ocol[:, :, 64:65],
                              onesT[:].rearrange("p (h o) -> p h o", o=1))
        vs_t.append(t)

    # out-projection weights stream in as slots free up
    wo1_t = load(p_w, wo1T, DIM, "w")
    wo2_t = load(p_w, wo2T, DIM, "w")

    # ---- K projection + attention, pipelined per head-pair ct ----
    val_t = []
    pending = []  # deferred normalize: (vt, po, ps_av, r)

    def finalize(p):
        vt, po, ps_av, r = p
        ps_b = ps_r.tile([64, QT], F32, tag="psr", name="psb")
        nc.tensor.matmul(ps_b[:], ones64[:], r[:], start=True, stop=True)
        nc.vector.tensor_copy(vt[po:po + 64, :], ps_av[0:64, :])
        nc.vector.tensor_mul(vt[po:po + 64, :], vt[po:po + 64, :], ps_b[:])

    for ct in range(IT):
        # ksT[c, k] for this head pair
        kst = p_ks.tile([128, DIM], F32R, tag="ks", name=f"ks{ct}")
        for nt in range(2):
            ps = ps_a.tile([128, QT], F32, tag="psa", name=f"pskp{ct}_{nt}")
            for ic in range(IT):
                nc.tensor.matmul(ps[:], wk_t[ic][:, ct * 128:(ct + 1) * 128],
                                 e_t[ic][:, nt * 512:(nt + 1) * 512],
                                 start=(ic == 0), stop=(ic == IT - 1))
            nc.vector.tensor_copy(kst[:, nt * 512:(nt + 1) * 512], ps[:])

        vt = p_t2k.tile([128, QT], F32R, tag="t2k", name=f"val{ct}")
        # scores + exp for both heads, kt-major: the sub=0 (rows 0:64) and
        # sub=1 (rows 64:128) matmuls sit on disjoint PE row-groups and
        # different PSUM banks, so adjacent pairs execute concurrently.
        exps = {0: [], 1: []}
        import os
        if os.environ.get("KERNEL_SC_INTERLEAVE", "1") == "1":
            order = [(kt, sub) for kt in range(IT) for sub in range(2)]
        else:
            order = [(kt, sub) for sub in range(2) for kt in range(IT)]
        for kt, sub in order:
            h = ct * 2 + sub
            po = sub * 64
            ps_s = ps_a.tile([128, QT], F32, tag="psa", name=f"pss{h}_{kt}")
            nc.tensor.matmul(ps_s[:], kst[po:po + 64, kt * 128:(kt + 1) * 128],
                             qs_t[ct][po:po + 64, :], start=True, stop=True)
            et = p_exp.tile([128, QT], BF16, tag="exp", name=f"ex{h}_{kt}")
            nc.scalar.activation(et[:], ps_s[:], EXP, scale=0.125)
            exps[sub].append(et)
        for sub in range(2):
            h = ct * 2 + sub
            po = sub * 64
            ps_av = ps_v.tile([128, QT], F32, tag="psv", name=f"psav{h}")
            for kt in range(IT):
                nc.tensor.matmul(ps_av[0:65, :], vs_t[kt][:, h * 65:(h + 1) * 65],
                                 exps[sub][kt][:], start=(kt == 0), stop=(kt == IT - 1))
            r = p_sm.tile([1, QT], F32R, tag="r", name=f"r{h}", bufs=2)
            with nc.allow_low_precision(reason="1/s rounded to f32r for bcast matmul"):
                nc.vector.reciprocal(r[:], ps_av[64:65, :])
            if pending:
                finalize(pending.pop(0))
            pending.append((vt, po, ps_av, r))
        val_t.append(vt)
    while pending:
        finalize(pending.pop(0))

    # ---- out1 = Wout_p . valsT ; out2 = Wout . out1T ----
    o1_t = []
    for ot in range(IT):
        ps = ps_a.tile([128, QT], F32, tag="psa", name=f"pso1_{ot}")
        for ic in range(IT):
            nc.tensor.matmul(ps[:], wo1_t[ic][:, ot * 128:(ot + 1) * 128], val_t[ic][:],
                             start=(ic == 0), stop=(ic == IT - 1))
        t = p_ks.tile([128, QT], F32R, tag="ks", name=f"o1_{ot}")
        nc.vector.tensor_copy(t[:], ps[:])
        o1_t.append(t)

    for ot in range(IT):
        ps = ps_a.tile([128, QT], F32, tag="psa", name=f"pso2_{ot}")
        for ic in range(IT):
            nc.tensor.matmul(ps[:], wo2_t[ic][:, ot * 128:(ot + 1) * 128], o1_t[ic][:],
                             start=(ic == 0), stop=(ic == IT - 1))
        t = p_qs.tile([128, QT], F32, tag="qs", name=f"ot{ot}")
        nc.vector.tensor_copy(t[:], ps[:])
        nc.sync.dma_start(out=outT[ot * 128:(ot + 1) * 128, :], in_=t[:])


def _prep(Wq, Wkv, Wout):
    """Host-side weight permutation/transposition (all fp32 numpy)."""
    d = np.arange(DK)
    h = np.arange(H)
    # perm[h*64+d] = d*16+h
    perm = (d[None, :] * H + h[:, None]).reshape(-1)
    Wk = Wkv[:DIM]
    Wv = Wkv[DIM:]
    wqT = np.ascontiguousarray(Wq[perm, :].T)
    wkT = np.ascontiguousarray(Wk[perm, :].T)
    wvT = np.ascontiguousarray(Wv[perm, :].T)
    wo1T = np.ascontiguousarray(Wout[:, perm].T)
    wo2T = np.ascontiguousarray(Wout.T)
    return wqT, wkT, wvT, wo1T, wo2T


def kernel(decoder_input, encoder_input, cross_mask, Wq, Wkv, Wout, _trace=False):
    import ml_dtypes
    decoder_input = np.asarray(decoder_input, dtype=np.float32)
    encoder_input = np.asarray(encoder_input, dtype=np.float32)
    Wq = np.asarray(Wq, dtype=np.float32)
    Wkv = np.asarray(Wkv, dtype=np.float32)
    Wout = np.asarray(Wout, dtype=np.float32)
    b, ql, _ = decoder_input.shape

    if "nc" not in _CACHE:
        _CACHE["nc"] = build_nc()
    nc = _CACHE["nc"]

    wqT, wkT, wvT, wo1T, wo2T = _prep(Wq, Wkv, Wout)
    pdt = ml_dtypes.bfloat16 if BF16_PROJ else np.float32
    wqT, wkT, wvT = wqT.astype(pdt), wkT.astype(pdt), wvT.astype(pdt)
    in_maps = []
    for core in range(8):
        bi, qh = divmod(core, 2)
        xT = np.ascontiguousarray(decoder_input[bi].T[:, qh * QT:(qh + 1) * QT]).astype(pdt)
        eT = np.ascontiguousarray(encoder_input[bi].T).astype(pdt)
        in_maps.append({"xT": xT, "eT": eT, "wqT": wqT, "wkT": wkT, "wvT": wvT,
                        "wo1T": wo1T, "wo2T": wo2T,
                        "onesA": np.ones((128, H), ml_dtypes.bfloat16),
                        "onesB": np.ones((1, 64), np.float32)})

    _CACHE["in_maps"] = in_maps
    res = run_bass_kernel_spmd(nc, in_maps, list(range(8)), trace=_trace)
    out = np.empty((b, ql, DIM), dtype=np.float32)
    for core in range(8):
        bi, qh = divmod(core, 2)
        out[bi, qh * QT:(qh + 1) * QT, :] = res.results[core]["outT"].T
    if _trace:
        _CACHE["last_result"] = res
    return out

